# revision 30
# baseline (speedup 1.0000x reference)
"""Multi-head attention Trainium2 kernel, 8-core SPMD.

Sharding: 16 (batch, head) pairs over 8 cores -> each core computes 2 heads
of one batch and returns a partial [N, D] output (bf16); host sums 4
partials per batch in fp32.

v3 dataflow (all transposed layouts prepared on HOST -- no xbar DMA
transposes, straight contiguous loads only):
  host:  xT[p, c, n] = x[b].T reshaped       [128, DC, N] bf16
  QT/KT = W.T @ xT   per q-slab              [128, N] bf16 (scale folded
                                             into Wq on host)
  Vn    = xT_v.T @ Wv  per m-chunk (natural) [128 m, mc, h, 65] bf16;
                                             col 64 = ones (rowsum trick)
  unit (qq, mc): both heads' S via concurrent PE row-tiles (K=64):
    S_h  = KT_h.T @ QT_h -> s2[:, h*512:]    [128 m, 1024] PSUM fp32
    P    = exp(s2)        one ACT op -> bf16 (softmax here is extremely
                          concentrated; fp8 P/V measured 5-7%% rel err)
    O_h += [V_h | 1].T @ P_h                 [65, 512] PSUM, accum over mc
  u-chain (split into fine steps trickled through the next quarter):
    r = O[64]; rb = ones.T @ r (bcast); un = O[0:64] * 1/rb
  out[q,:] = un2.T @ Wp (head sum via 128-contraction), bf16 store
"""

import os
import sys

import numpy as np

sys.path.insert(0, "/opt/trn_rl_repo")

import ml_dtypes
from contextlib import ExitStack

import concourse.bass as bass
import concourse.mybir as mybir
import concourse.tile as tile
from concourse import bacc
from concourse.bass_utils import run_bass_kernel_spmd

B, N, D, H, HS = 2, 2048, 512, 8, 64
NCORES = 8
BF16 = mybir.dt.bfloat16
FP32 = mybir.dt.float32
FP8 = mybir.dt.float8e4
nbf16 = ml_dtypes.bfloat16
nfp8 = ml_dtypes.float8_e4m3

DC = D // 128  # 4 d-chunks
MC = N // 128  # 16 m-chunks
JP = MC // 2  # 8 m-chunk pairs (fp8 DoubleRow PV)
QQ = 4  # q quarters
QV = N // QQ  # 512 q per quarter
SLAB = 512  # proj/dma slab width
PV_LAG = 1  # units between exp and PV in the PE queue


def build_nc(finalize=True, repeat=1):
    nc = bacc.Bacc()
    NS = N // SLAB
    xq = nc.dram_tensor("xq", [128, NS, DC, SLAB], BF16, kind="ExternalInput")
    xk = nc.dram_tensor("xk", [128, NS, DC, SLAB], BF16, kind="ExternalInput")
    xv = nc.dram_tensor("xv", [128, NS, DC, SLAB], BF16, kind="ExternalInput")
    wall = nc.dram_tensor("wall", [128, 4 * 512], BF16, kind="ExternalInput")
    out = nc.dram_tensor("out", [N, D], BF16, kind="ExternalOutput")

    with tile.TileContext(nc) as tc, ExitStack() as ctx:
        consts = ctx.enter_context(tc.tile_pool(name="consts", bufs=1))
        xt_pool = ctx.enter_context(tc.tile_pool(name="xt", bufs=1))
        kq_pool = ctx.enter_context(tc.tile_pool(name="kq", bufs=1))
        pt_pool = ctx.enter_context(tc.tile_pool(name="pt", bufs=6))
        un_pool = ctx.enter_context(tc.tile_pool(name="un", bufs=2))
        rs_pool = ctx.enter_context(tc.tile_pool(name="rs", bufs=2))
        rb_pool = ctx.enter_context(tc.tile_pool(name="rb", bufs=4))
        ob_pool = ctx.enter_context(tc.tile_pool(name="ob", bufs=3))
        psA = ctx.enter_context(tc.tile_pool(name="psA", bufs=2, space="PSUM"))
        psO = ctx.enter_context(tc.tile_pool(name="psO", bufs=4, space="PSUM"))

        for _rep in range(repeat):
            # all weights in ONE packed tile / one DMA (issued on the
            # gpsimd SWDGE queue; k/q slabs go on the sync/scalar HWDGE
            # queues so the three input streams issue in parallel)
            wall_s = consts.tile([128, 4 * 512], BF16, tag="wall_s")
            nc.sync.dma_start(out=wall_s[:], in_=wall[:])

            def w_slice(name, dc):
                off = {"q": 0, "k": 512, "v": 1024}[name] + dc * 128
                return wall_s[:, off : off + 128]

            wp_s = wall_s[:, 1536:2048]

            # Vn: [128 m, mc, head, 65]; col HS = ones (rowsum trick)
            vn = consts.tile([128, MC, 2, HS + 1], BF16, tag="vn")
            nc.gpsimd.memset(vn[:, :, :, HS : HS + 1], 1.0)
            # rowsum-broadcast ones row lives at partition HS (=64) so the
            # lhsT/rhs base partitions match
            ones_row = consts.tile([HS + 1, HS], BF16, tag="ones_row")
            nc.gpsimd.memset(ones_row[HS : HS + 1, :], 1.0)
            # warm the ACT exp table while DMAs stream
            warm = consts.tile([1, 1], BF16, tag="warm")
            nc.scalar.activation(
                warm[:], ones_row[HS : HS + 1, 0:1],
                mybir.ActivationFunctionType.Exp,
            )

            # X (pre-transposed on host, slab-major so each DMA moves
            # 4KB-contiguous per-partition lines -> 128 descriptors/slab):
            # ordered so the first attention units are gated by as little
            # DMA as possible; trailing slabs merged into bigger transfers
            xts = {
                "q": xt_pool.tile([128, NS, DC, SLAB], BF16, tag="xt_q", name="xt_q"),
                "k": xt_pool.tile([128, NS, DC, SLAB], BF16, tag="xt_k", name="xt_k"),
                "v": xt_pool.tile([128, NS, DC, SLAB], BF16, tag="xt_v", name="xt_v"),
            }
            # single sync-issued stream in strict priority order (the 16
            # SDMA engines round-robin across queues, so spreading across
            # issue queues just slows the critical k0/q0 prefix)
            for name, j0, j1 in [
                ("k", 0, 1), ("q", 0, 1), ("v", 0, 1), ("k", 1, 2),
                ("v", 1, 2), ("k", 2, 4), ("v", 2, 4), ("q", 1, 4),
            ]:
                dram = {"q": xq, "k": xk, "v": xv}[name]
                nc.sync.dma_start(out=xts[name][:, j0:j1], in_=dram[:, j0:j1])

            kt2 = kq_pool.tile([128, N], BF16, tag="kt2", name="kt2")
            qt2 = kq_pool.tile([128, N], BF16, tag="qt2", name="qt2")
            dst2 = {"k": kt2, "q": qt2}

            def proj_slab(name, j):
                pr = psA.tile([128, SLAB], FP32, tag="ps", name="pr")
                for dc in range(DC):
                    nc.tensor.matmul(
                        pr[:],
                        w_slice(name, dc),
                        xts[name][:, j, dc, :],
                        start=(dc == 0),
                        stop=(dc == DC - 1),
                    )
                nc.vector.tensor_copy(
                    dst2[name][:, j * SLAB : (j + 1) * SLAB], pr[:]
                )

            def proj_slab2(name, j):
                # two 512-col slabs sharing each dc's stationary load
                pr = psA.tile([128, 2 * SLAB], FP32, tag="ps", name="pr")
                for dc in range(DC):
                    for s in range(2):
                        nc.tensor.matmul(
                            pr[:, s * SLAB : (s + 1) * SLAB],
                            w_slice(name, dc),
                            xts[name][:, j + s, dc, :],
                            start=(dc == 0),
                            stop=(dc == DC - 1),
                        )
                nc.vector.tensor_copy(
                    dst2[name][:, j * SLAB : (j + 2) * SLAB], pr[:]
                )


            def proj_parts(name, j, wide=False):
                # split a slab projection across two deferred slots so no
                # single slot injects more than ~4 matmuls into the stream
                st = {}
                nsl = 2 if wide else 1

                def part_a():
                    st["pr"] = psA.tile(
                        [128, nsl * SLAB], FP32, tag="ps", name="pr"
                    )
                    for dc in (0, 1):
                        for s in range(nsl):
                            nc.tensor.matmul(
                                st["pr"][:, s * SLAB : (s + 1) * SLAB],
                                w_slice(name, dc),
                                xts[name][:, j + s, dc, :],
                                start=(dc == 0),
                                stop=False,
                            )

                def part_b():
                    for dc in (2, 3):
                        for s in range(nsl):
                            nc.tensor.matmul(
                                st["pr"][:, s * SLAB : (s + 1) * SLAB],
                                w_slice(name, dc),
                                xts[name][:, j + s, dc, :],
                                start=False,
                                stop=(dc == DC - 1),
                            )
                    nc.vector.tensor_copy(
                        dst2[name][:, j * SLAB : (j + nsl) * SLAB], st["pr"][:]
                    )

                return [part_a, part_b]

            def vproj_half(jh):
                # V in natural [m, hs2] orientation: 2 m-chunks per item
                vp = psA.tile([128, 256], FP32, tag="ps", name="vp")
                for m2 in range(2):
                    mc = jh * 2 + m2
                    for dc in range(DC):
                        nc.tensor.matmul(
                            vp[:, m2 * 128 : (m2 + 1) * 128],
                            xts["v"][:, mc // 4, dc, (mc % 4) * 128 : (mc % 4 + 1) * 128],
                            w_slice("v", dc),
                            start=(dc == 0),
                            stop=(dc == DC - 1),
                        )
                nc.vector.tensor_copy(
                    vn[:, jh * 2 : (jh + 1) * 2, :, 0:HS],
                    vp[:].rearrange("p (m h c) -> p m h c", m=2, h=2),
                )

            def emit_final(qq_, un_t, c, last=False):
                f_ps = psA.tile([128, D], FP32, tag="ps", name="f_ps")
                nc.tensor.matmul(
                    f_ps[:],
                    un_t[:, c * 128 : (c + 1) * 128],
                    wp_s[:],
                    start=True,
                    stop=True,
                )
                ob = ob_pool.tile([128, D], BF16, tag="ob", name="ob")
                if last:
                    # the ACT engine is idle after the final exp; use it for
                    # the tail casts so the DVE chain isn't the critical path
                    nc.scalar.copy(ob[:], f_ps[:])
                else:
                    nc.vector.tensor_copy(ob[:], f_ps[:])
                (nc.scalar if last else nc.sync).dma_start(
                    out=out[qq_ * QV + c * 128 : qq_ * QV + (c + 1) * 128, :],
                    in_=ob[:],
                )

            def tail_steps(qq_, o_ps_, un_t, last=False):
                # r = rowsums (row 64 of o); broadcast to 64 partitions via
                # one ones.T @ [r_h0|r_h1] matmul; un = o[0:64] * 1/rb.
                # Split into small steps so each PE op only waits on DVE
                # work from >=1 slot ago.
                r_sb = rs_pool.tile([HS + 1, 2, QV], BF16, tag="r")
                rb_sb = rb_pool.tile([HS, 2, QV], FP32, tag="rb", name="rb_sb")

                def cpy():
                    for h in range(2):
                        if last:
                            nc.scalar.copy(
                                r_sb[HS : HS + 1, h, :], o_ps_[h][HS : HS + 1, :]
                            )
                        else:
                            nc.vector.tensor_copy(
                                r_sb[HS : HS + 1, h, :], o_ps_[h][HS : HS + 1, :]
                            )

                rb_ps = [None]

                def rbmm():
                    # one MM per head (PSUM bank limit: 512 fp32 out free)
                    rb_ps[0] = psA.tile([HS, 2 * QV], FP32, tag="ps", name="rb_ps")
                    for h in range(2):
                        nc.tensor.matmul(
                            rb_ps[0][:, h * QV : (h + 1) * QV],
                            ones_row[HS : HS + 1, :],
                            r_sb[HS : HS + 1, h, :],
                            start=True,
                            stop=True,
                        )

                def recip(lo, hi):
                    for h in range(2):
                        nc.vector.reciprocal_approx_fast(
                            rb_sb[:, h, lo:hi], rb_ps[0][:, h * QV + lo : h * QV + hi]
                        )

                def mul(h, lo, hi):
                    nc.vector.tensor_mul(
                        un_t[HS * h : HS * h + HS, lo:hi],
                        o_ps_[h][0:HS, lo:hi],
                        rb_sb[:, h, lo:hi],
                    )

                if last:
                    # half-granularity so the first final MMs start while
                    # the second half of the normalization still runs
                    HF = QV // 2
                    return [
                        cpy,
                        rbmm,
                        lambda: recip(0, HF),
                        lambda: (mul(0, 0, HF), mul(1, 0, HF)),
                        lambda: (recip(HF, QV), emit_final(qq_, un_t, 0, last)),
                        lambda: (mul(0, HF, QV), mul(1, HF, QV)),
                        lambda: emit_final(qq_, un_t, 1, last),
                        lambda: emit_final(qq_, un_t, 2, last),
                        lambda: emit_final(qq_, un_t, 3, last),
                    ]
                return [
                    cpy,
                    lambda: (rbmm(), recip(0, QV)),
                    lambda: mul(0, 0, QV),
                    lambda: mul(1, 0, QV),
                    lambda: emit_final(qq_, un_t, 0, last),
                    lambda: emit_final(qq_, un_t, 1, last),
                    lambda: emit_final(qq_, un_t, 2, last),
                    lambda: emit_final(qq_, un_t, 3, last),
                ]

            # K slab 0 + Q slab 0 gate the first S; everything else is
            # trickled through the attention stream's deferred slots
            proj_slab("k", 0)
            proj_slab("q", 0)
            deferred = (
                proj_parts("k", 1)
                + [lambda: vproj_half(0), lambda: vproj_half(1)]
                + proj_parts("k", 2, wide=True)
                + [
                    lambda: vproj_half(2),
                    lambda: vproj_half(3),
                    lambda: vproj_half(4),
                    lambda: vproj_half(5),
                ]
                + proj_parts("q", 1)
                + [lambda: vproj_half(6), lambda: vproj_half(7)]
            )

            for qq in range(QQ):
                o_ps = [
                    psO.tile([HS + 1, QV], FP32, tag="o", name=f"o{h}")
                    for h in range(2)
                ]
                un2 = un_pool.tile([128, QV], BF16, tag="un")

                def pv(mc, p_sb, o_ps_=o_ps):
                    for h in range(2):
                        nc.tensor.matmul(
                            o_ps_[h][:],
                            vn[:, mc, h, :],
                            p_sb[:, h * QV : (h + 1) * QV],
                            start=(mc == 0),
                            stop=(mc == MC - 1),
                        )

                pend = []
                for mc in range(MC):
                    s2 = psA.tile([128, 1024], FP32, tag="ps", name="s2")
                    for h in range(2):
                        nc.tensor.matmul(
                            s2[:, h * QV : (h + 1) * QV],
                            kt2[h * HS : (h + 1) * HS, mc * 128 : (mc + 1) * 128],
                            qt2[h * HS : (h + 1) * HS, qq * QV : (qq + 1) * QV],
                            start=True,
                            stop=True,
                            tile_position=(h * HS, 0),
                        )
                    p_sb = pt_pool.tile([128, 1024], BF16, tag="p", name="p_sb")
                    nc.scalar.activation(
                        p_sb[:], s2[:], mybir.ActivationFunctionType.Exp
                    )
                    if deferred:
                        deferred.pop(0)()
                    pend.append((mc, p_sb))
                    lag = PV_LAG
                    if qq == QQ - 1 and mc >= MC - 3:
                        lag = 1
                    while len(pend) > lag:
                        pv(*pend.pop(0))

                if qq < QQ - 1:
                    # trickle the prior quarter's trailing PVs,
                    # normalization + output projection through the next
                    # quarter's stream (PVs must precede tail in the queue)
                    for e in pend:
                        deferred.append(lambda e=e, pvf=pv: pvf(*e))
                    pend.clear()
                    deferred.extend(tail_steps(qq, o_ps, un2))
                    if qq == 0:
                        deferred.extend(proj_parts("q", 2, wide=True))
                else:
                    for e in pend:
                        pv(*e)
                    for step in tail_steps(qq, o_ps, un2, last=True):
                        step()
    if finalize:
        nc.finalize()
    return nc


_NC_CACHE = None


def _get_nc():
    global _NC_CACHE
    if _NC_CACHE is None:
        _NC_CACHE = build_nc()
    return _NC_CACHE


def _prep_xt(x, dt):
    # [N, D] fp32 -> [128, NS, DC, SLAB] slab-major:
    # xt[p, j, c, n'] = x[j*SLAB + n', c*128 + p]
    return np.ascontiguousarray(
        x.reshape(N // SLAB, SLAB, DC, 128).transpose(3, 0, 2, 1)
    ).astype(dt)


def _prep_w(w2, dt):
    # [D, 128] -> [128, DC, 128] with w[p, c, h] = w2[c*128+p, h]
    return np.ascontiguousarray(
        w2.reshape(DC, 128, 128).transpose(1, 0, 2)
    ).astype(dt)


def make_in_maps(inputs):
    query = np.asarray(inputs["query"], np.float32)
    key = np.asarray(inputs["key"], np.float32)
    value = np.asarray(inputs["value"], np.float32)
    Wq = np.asarray(inputs["Wq"], np.float32) / np.sqrt(np.float32(HS))
    Wk = np.asarray(inputs["Wk"], np.float32)
    Wv = np.asarray(inputs["Wv"], np.float32)
    Wp = np.asarray(inputs["Wp"], np.float32)

    in_maps = []
    for c in range(NCORES):
        b = c // 4
        h0 = 2 * (c % 4)
        w_all = np.concatenate(
            [
                _prep_w(
                    np.concatenate([W[h0], W[h0 + 1]], axis=1), np.float32
                ).reshape(128, DC * 128)
                for W in (Wq, Wk, Wv)
            ]
            + [np.concatenate([Wp[h0], Wp[h0 + 1]], axis=0)],
            axis=1,
        ).astype(nbf16)
        in_maps.append(
            {
                "xq": _prep_xt(query[b], nbf16),
                "xk": _prep_xt(key[b], nbf16),
                "xv": _prep_xt(value[b], nbf16),
                "wall": np.ascontiguousarray(w_all),
            }
        )
    return in_maps


def kernel(query, key, value, Wq, Wk, Wv, Wp):
    in_maps = make_in_maps(
        dict(query=query, key=key, value=value, Wq=Wq, Wk=Wk, Wv=Wv, Wp=Wp)
    )
    nc = _get_nc()
    res = run_bass_kernel_spmd(nc, in_maps, list(range(NCORES)))
    out = np.zeros((B, N, D), np.float32)
    for c in range(NCORES):
        out[c // 4] += np.asarray(res.results[c]["out"], np.float32)
    return out


if __name__ == "__main__":
    d = np.load("/root/problem/work/ref.npz")
    got = kernel(
        d["query"], d["key"], d["value"], d["Wq"], d["Wk"], d["Wv"], d["Wp"]
    )
    exp = d["expected"]
    rel = np.linalg.norm(got - exp) / np.linalg.norm(exp)
    print("Relative error:", rel)
